# revision 6
# baseline (speedup 1.0000x reference)
"""BiMPM matching-layer kernel for 8 Trainium2 NeuronCores.

Data-parallel over the batch axis: each of the 8 cores gets 8 of the 64
batch elements (full hidden=100 and seq=384 on every core). Weights are
replicated (host-squared into the forms the device needs).

Per (batch, direction): m1 (cosine vs last q timestep), m2 (max pairwise
weighted cosine), m3 (cosine vs attention-mean of q), m4 (cosine vs
argmax-attended q), for l in {0,1}. The backward direction reuses the
*forward* w3/w4 tables (reference bug preserved). Outputs: 8 tensors of
shape (384, 64, 2), order m1f,m1b,m2f,m2b,m3f,m3b,m4f,m4b.
"""

import os
import sys

sys.path.insert(0, "/opt/trn_rl_repo")

import numpy as np

H, B, S, L = 100, 64, 384, 2
NCORES = 8
BC = B // NCORES  # 8 batches per core
NT = S // 128  # 3 tiles of 128 along seq

_COMPILED = {}


def _build_program(c_fp32: bool):
    """Builds the single-core SPMD Bass program (same on all 8 cores)."""
    import concourse.bacc as bacc
    import concourse.bass as bass
    import concourse.mybir as mybir
    import concourse.tile as tile
    from concourse import masks
    from concourse.bass_types import AP

    dt = mybir.dt
    f32 = dt.float32
    f32r = dt.float32r
    AF = mybir.ActivationFunctionType
    ALU = mybir.AluOpType
    AX = mybir.AxisListType

    nc = bacc.Bacc("TRN2", target_bir_lowering=False, debug=False)

    ins = {}
    for nm in ("p_f", "p_b", "q_f", "q_b"):
        ins[nm] = nc.dram_tensor(nm, [H, BC, S], f32r,
                                 kind="ExternalInput")
    WSB = nc.dram_tensor("WSB", [H, 18], f32r, kind="ExternalInput")
    U1C = nc.dram_tensor("U1C", [H, 4], f32r, kind="ExternalInput")
    U2C = nc.dram_tensor("U2C", [H, 4], f32, kind="ExternalInput")
    U3C = nc.dram_tensor("U3C", [H, 4], f32, kind="ExternalInput")
    UBC34 = nc.dram_tensor("UBC34", [128, 800], f32, kind="ExternalInput")
    SEL = nc.dram_tensor("SEL", [9, 384], f32r, kind="ExternalInput")
    IDN = nc.dram_tensor("IDN", [128, 128], f32r, kind="ExternalInput")
    SELF_ = nc.dram_tensor("SELF", [9, 128], f32, kind="ExternalInput")
    WSBF = nc.dram_tensor("WSBF", [H, 18], f32, kind="ExternalInput")
    DBG = nc.dram_tensor("dbg", [128, 48], mybir.dt.float32,
                         kind="ExternalOutput")
    DBG2 = nc.dram_tensor("dbg2", [128, 390], mybir.dt.float32,
                          kind="ExternalOutput")
    outs = {}
    for nm in ("m1f", "m1b", "m2f", "m2b", "m3f", "m3b", "m4f", "m4b"):
        outs[nm] = nc.dram_tensor(nm, [S, BC, L], f32, kind="ExternalOutput")
    # per-pair DRAM scratch for the gather source
    GTs = [nc.dram_tensor(f"GT{i}", [S, 204], f32) for i in range(2 * BC)]

    CDT = f32 if c_fp32 else f32r  # dtype for the argmax-critical matmul

    def ap3(t, off, pattern):
        """AP on tile t: partition dim + explicit free-dim [step,count]s."""
        base = t[:, 0:1]
        part = list(base.ap[0])
        return AP(base.tensor, base.offset + off,
                  [part] + [list(x) for x in pattern])

    def r(ap):
        return ap.bitcast(f32r)

    def f(ap):
        return ap.bitcast(f32)

    def rep2(ap, n):
        """(P, n) AP -> (P, 2, n) stride-0 repeat."""
        return AP(ap.tensor, ap.offset,
                  [list(ap.ap[0]), [0, 2], [1, n]])

    with tile.TileContext(nc) as tc:
        con = tc.alloc_tile_pool(name="con", bufs=1)
        big = tc.alloc_tile_pool(name="big", bufs=2)
        sml = tc.alloc_tile_pool(name="sml", bufs=2)
        stg = tc.alloc_tile_pool(name="stg", bufs=1)
        ps = tc.alloc_tile_pool(name="ps", bufs=1, space="PSUM")

        # ---- constants -----------------------------------------------
        idn = con.tile([128, 128], f32r, tag="idn")
        nc.sync.dma_start(idn[:], IDN.ap())
        onesb = con.tile([128, 128], f32, tag="onesb")
        nc.vector.memset(onesb[0:1, :], 1.0)
        ones_row = onesb[0:1, :]
        wsbf_t = con.tile([128, 18], f32r, tag="wsb")
        nc.sync.dma_start(wsbf_t[0:H, :], WSB.ap())
        wsb = wsbf_t[0:H, :]
        wsbff_t = con.tile([128, 18], f32, tag="wsbff")
        nc.sync.dma_start(wsbff_t[0:H, :], WSBF.ap())
        wsbff = wsbff_t[0:H, :]
        u1c_t = con.tile([128, 4], f32r, tag="u1c")
        nc.sync.dma_start(u1c_t[0:H, :], U1C.ap())
        u1c = u1c_t[0:H, :]
        u2c_t = con.tile([128, 4], f32, tag="u2c")
        nc.sync.dma_start(u2c_t[0:H, :], U2C.ap())
        u2c = u2c_t[0:H, :]
        u3c_t = con.tile([128, 4], f32, tag="u3c")
        nc.sync.dma_start(u3c_t[0:H, :], U3C.ap())
        u3c = u3c_t[0:H, :]
        ubc34 = con.tile([128, 800], f32, tag="ubc34")
        nc.sync.dma_start(ubc34[:], UBC34.ap())
        sel_t = con.tile([128, 384], f32r, tag="sel")
        nc.sync.dma_start(sel_t[0:9, :], SEL.ap())
        sel = sel_t[0:9, :]
        self_t = con.tile([128, 128], f32, tag="selfp")
        nc.sync.dma_start(self_t[0:9, :], SELF_.ap())
        selfp = self_t[0:9, :]
        zeros8 = con.tile([128, 8], f32, tag="zeros8")
        nc.vector.memset(zeros8[:], 0.0)
        # output staging: col = out_idx*48 + it*16 + b*2 + l
        stgt = stg.tile([128, 384], f32, tag="stgt")

        for di, d in enumerate(("f", "b")):
            P_in, Q_in = ins["p_" + d], ins["q_" + d]
            for b in range(BC):
                GT = GTs[di * BC + b]
                # ---- phase A: load, norms, scaled q variants ---------
                psb_t = big.tile([128, S], f32r, tag="psb")
                qsb_t = big.tile([128, S], f32r, tag="qsb")
                psb, qsb = psb_t[0:H, :], qsb_t[0:H, :]
                nc.sync.dma_start(psb[:], P_in.ap()[:, b, :])
                nc.sync.dma_start(qsb[:], Q_in.ap()[:, b, :])
                psbf_t = big.tile([128, S], f32, tag="psbf")
                psbf = psbf_t[0:H, :]
                nc.sync.dma_start(psbf[:], f(P_in.ap()[:, b, :]))
                # unrounded q: the f32r DMA rounds to tf32, which is too
                # coarse for the m3 sum-sign and the m4 argmax.
                qsbf_t = big.tile([128, S], f32, tag="qsbf")
                qsbf = qsbf_t[0:H, :]
                nc.sync.dma_start(qsbf[:], f(Q_in.ap()[:, b, :]))
                psq_t = big.tile([128, S], f32r, tag="psq")
                qsq_t = big.tile([128, S], f32, tag="qsq")
                psq, qsq = psq_t[0:H, :], qsq_t[0:H, :]
                nc.scalar.activation(psq[:], f(psb[:]), AF.Square)
                nc.vector.tensor_tensor(qsq[:], qsbf[:], qsbf[:],
                                        op=ALU.mult)

                pnqn = ps.tile([9, 896], f32, tag="pnqn")
                wslice = r(wsb[:, di * 9 : di * 9 + 9])
                nc.tensor.matmul(pnqn[:, 0:S], wslice, r(psq[:]),
                                 start=True, stop=True)
                nc.tensor.matmul(pnqn[:, 512 : 512 + S],
                                 wsbff[:, di * 9 : di * 9 + 9], qsq[:],
                                 start=True, stop=True)
                ipqn_t = sml.tile([128, 2 * S], f32, tag="ipqn")
                rscr_t = sml.tile([128, 2 * S], f32, tag="rscr")
                ipqn, rscr = ipqn_t[0:9, :], rscr_t[0:9, :]
                from concourse.dve_ops import RECIPROCAL_APPROX_NR
                nc.scalar.activation(ap3(rscr, 0, [[S, 2], [1, S]]),
                                     ap3(pnqn, 0, [[512, 2], [1, S]]),
                                     AF.Sqrt)
                nc.vector.reciprocal_approx_fast(out=ipqn[:], in_=rscr[:])
                nc.vector._custom_dve(RECIPROCAL_APPROX_NR, out=ipqn[:],
                                      in0=rscr[:], in1=ipqn[:], s0=2.0)
                nc.vector._custom_dve(RECIPROCAL_APPROX_NR, out=ipqn[:],
                                      in0=rscr[:], in1=ipqn[:], s0=2.0)
                ipn = ipqn[:, 0:S]
                iqn = ipqn[:, S : 2 * S]
                # rsqrt Newton polish on the q rows: kills the ACT-LUT
                # sqrt error that perturbs argmax columns.
                nra = rscr[:, 0:S]
                nc.vector.tensor_tensor(nra[:], pnqn[:, 512 : 512 + S],
                                        iqn[:], op=ALU.mult)
                nc.vector.tensor_tensor(nra[:], nra[:], iqn[:], op=ALU.mult)
                nc.vector.tensor_scalar(nra[:], nra[:], -0.5, 1.5,
                                        op0=ALU.mult, op1=ALU.add)
                nc.vector.tensor_tensor(iqn[:], iqn[:], nra[:], op=ALU.mult)

                # misc PSUM bank: ipn transposed + m1 scale + iqn4 transp.
                misc = ps.tile([128, 512], f32, tag="misc")
                for it in range(NT):
                    nc.tensor.transpose(misc[:, it * 9 : it * 9 + 9],
                                        ipn[:, it * 128 : (it + 1) * 128],
                                        f(idn[:9, :9]))
                # nq_l^2 = sum(u1_l * qlast^2) -> (1,2) at cols 32:34
                nc.tensor.matmul(misc[0:1, 32:34], f(qsq[:, S - 1 : S]),
                                 f(u1c[:, di * 2 : di * 2 + 2]),
                                 start=True, stop=True)
                nqr = sml.tile([128, 2], f32, tag="nqr")
                nc.scalar.activation(nqr[0:1, :], misc[0:1, 32:34], AF.Sqrt)
                nc.vector.reciprocal(nqr[0:1, :], nqr[0:1, :])
                nc.tensor.matmul(misc[:, 40:42], ones_row[:],
                                 nqr[0:1, :], start=True, stop=True)
                # inverse den4 columns: transpose iqn rows 7:9 per tile
                for it in range(NT):
                    nc.tensor.transpose(
                        misc[:, 64 + it * 3 : 64 + it * 3 + 3],
                        iqn[0:3, it * 128 : (it + 1) * 128],
                        f(idn[:3, :3]))
                ipns = sml.tile([128, 27], f32, tag="ipns")
                nc.vector.tensor_copy(ipns[:], misc[:, 0:27])
                nq4 = sml.tile([128, 4], f32, tag="nq4")
                nc.vector.tensor_copy(nq4[:, 0:2], misc[:, 40:42])
                nc.vector.memset(nq4[:, 2:4], 1.0)
                iq4t = sml.tile([128, 9], f32, tag="iq4t")
                nc.vector.tensor_copy(iq4t[:], misc[:, 64:73])

                # broadcasts of 1/normq rows, one at a time (tag shares
                # the mega slot: used before any mega tile of this pair)
                qn_t = big.tile([128, S + 2], f32, tag="qn")
                q2n_t = big.tile([128, 2 * S], f32r, tag="q2n")
                q2a_t = big.tile([128, 2 * S], f32, tag="q2a")
                qn, q2n, q2a = qn_t[0:H, :], q2n_t[0:H, :], q2a_t[0:H, :]

                nc.scalar.activation(q2a[:, 0:S], f(qsb[:]), AF.Copy,
                                     scale=u2c[:, di * 2 : di * 2 + 1])
                nc.scalar.activation(q2a[:, S : 2 * S], f(qsb[:]), AF.Copy,
                                     scale=u2c[:, di * 2 + 1 : di * 2 + 2])
                iqr5_t = sml.tile([128, S], f32r, tag="iqr5")
                iqr5 = iqr5_t[0:5, :]
                nc.vector.tensor_copy(iqr5[:], iqn[0:5, :])
                for k, (src, dst) in enumerate(
                        ((qsbf, qn), (q2a, q2n), (q2a, q2n))):
                    if k == 0:
                        bcq = sml.tile([128, S], f32, tag="bcq")
                        nc.gpsimd.partition_broadcast(bcq[0:H, :],
                                                      iqn[0:1, :])
                        nc.vector.tensor_tensor(dst[:, 0:S], f(src[:, 0:S]),
                                                bcq[0:H, :], op=ALU.mult)
                    else:
                        bc = ps.tile([128, 512], f32, tag="mega")
                        nc.tensor.matmul(
                            bc[0:H, 0:S],
                            sel[0:5, k * 128 : k * 128 + H],
                            iqr5[:], start=True, stop=True)
                        off = (k - 1) * S
                        nc.vector.tensor_tensor(dst[:, off : off + S],
                                                src[:, off : off + S],
                                                bc[0:H, 0:S], op=ALU.mult)
                nc.vector.tensor_tensor(
                    qn[:, S : S + 2],
                    qsbf[:, S - 1 : S].to_broadcast([H, 2]),
                    f(u1c[:, di * 2 : di * 2 + 2]),
                    op=ALU.mult)

                # ---- phase B: i-layout matmuls + fused maxes ---------
                fsc = sml.tile([128, 48], f32, tag="fsc")
                idx8 = [sml.tile([128, 8], dt.uint32, tag=f"idx{it}", name=f"idx{it}")
                        for it in range(NT)]
                for it in range(NT):
                    mega = ps.tile([128, 1536], f32, tag="mega")
                    lhs = r(psb[:, it * 128 : (it + 1) * 128])
                    nc.tensor.matmul(mega[:, 0:S], lhs,
                                     r(q2n[:, 0:S]), start=True, stop=True)
                    nc.tensor.matmul(mega[:, 512 : 512 + S], lhs,
                                     r(q2n[:, S : 2 * S]),
                                     start=True, stop=True)
                    nc.tensor.matmul(
                        mega[:, 1024 : 1024 + S + 2],
                        psbf[:, it * 128 : (it + 1) * 128].bitcast(CDT),
                        qn[:].bitcast(CDT), start=True, stop=True)
                    nc.vector.reduce_max(
                        fsc[:, it * 16 + 8 : it * 16 + 11],
                        ap3(mega, 0, [[512, 3], [1, S]]), axis=AX.X)
                    mx8 = sml.tile([128, 8], f32, tag="mx8")
                    nc.vector.tensor_scalar(
                        mx8[:], zeros8[:],
                        fsc[:, it * 16 + 10 : it * 16 + 11], None,
                        op0=ALU.add)
                    nc.vector.max_index(
                        idx8[it][:], mx8[:],
                        mega[:, 1024 : 1024 + S])
                    nc.vector.tensor_copy(
                        fsc[:, it * 16 + 11 : it * 16 + 13],
                        mega[:, 1024 + S : 1024 + S + 2])
                    nc.vector.reduce_sum(
                        fsc[:, it * 16 + 14 : it * 16 + 15],
                        mega[:, 1024 : 1024 + S], axis=AX.X)

                # ---- phase C: T-layout -------------------------------
                cts = [big.tile([128, S], f32r, tag=f"cts{jt}", name=f"cts{jt}")
                       for jt in range(NT)]
                hmrhs = [big.tile([128, 512], f32r, tag=f"hmrhs{jt}", name=f"hmrhs{jt}")
                         for jt in range(NT)]
                ptse = [sml.tile([128, 400], f32, tag=f"ptse{jt}", name=f"ptse{jt}")
                        for jt in range(NT)]
                p4u = [sml.tile([128, 200], f32, tag=f"p4u{jt}", name=f"p4u{jt}")
                       for jt in range(NT)]
                for jt in range(NT):
                    ctp = ps.tile([128, S], f32, tag="ctp")
                    nc.tensor.matmul(
                        ctp[:], qsb[:, jt * 128 : (jt + 1) * 128],
                        psb[:], start=True, stop=True)
                    nc.vector.tensor_scalar(
                        cts[jt][:], ctp[:],
                        iq4t[:, jt * 3 : jt * 3 + 1], None,
                        op0=ALU.mult)
                    tqp = ps.tile([128, 224], f32, tag="misc")
                    nc.tensor.transpose(tqp[:, 0:H],
                                        f(qsb[:, jt * 128 : (jt + 1) * 128]),
                                        f(idn[:H, :H]))
                    nc.tensor.transpose(tqp[:, 112 : 112 + H],
                                        f(psb[:, jt * 128 : (jt + 1) * 128]),
                                        f(idn[:H, :H]))

                    nc.vector.tensor_copy(hmrhs[jt][:, 0:H],
                                          tqp[:, 0:H])
                    nc.vector.memset(f(hmrhs[jt][:, 100:101]), 1.0)
                    nc.vector.memset(f(hmrhs[jt][:, 501:512]), 0.0)
                    gsrc = sml.tile([128, 204], f32, tag="gsrc")
                    nc.vector.tensor_copy(gsrc[:, 0:H], tqp[:, 0:H])
                    nc.vector.tensor_copy(gsrc[:, H : 2 * H],
                                          tqp[:, 0:H])
                    nc.vector.tensor_copy(ptse[jt][:, 0:H],
                                          tqp[:, 112 : 112 + H])
                    nc.vector.tensor_copy(ptse[jt][:, H : 2 * H],
                                          tqp[:, 112 : 112 + H])
                    # q3uT_l twice: cols 101+100l:201+100l & 301+100l:..
                    for l in range(2):
                        nc.vector.tensor_tensor(
                            ap3(hmrhs[jt], 101 + 100 * l,
                                [[200, 2], [1, H]]).bitcast(f32r),
                            gsrc[:, 0 : 2 * H],
                            ubc34[:, 400 + 200 * l : 600 + 200 * l],
                            op=ALU.mult)
                    nc.vector.tensor_copy(gsrc[:, 200:202],
                                          iq4t[:, jt * 3 + 1 : jt * 3 + 3])
                    nc.vector.memset(gsrc[:, 202:204], 0.0)
                    nc.sync.dma_start(
                        GT.ap()[jt * 128 : (jt + 1) * 128, :], gsrc[:])
                    # p4uT pack = [pT*u4_0 | pT*u4_1]
                    nc.vector.tensor_tensor(
                        p4u[jt][:], ptse[jt][:, 0 : 2 * H],
                        ubc34[:, 200:400], op=ALU.mult)

                # ---- phase D: hm matmuls, gather, dot products -------
                pscr = sml.tile([128, 400], f32, tag="pscr")
                for it in range(NT):
                    hmx = ps.tile([128, 512], f32, tag="hmx")
                    for jt in range(NT):
                        nc.tensor.matmul(
                            hmx[:, 0:502],
                            r(cts[jt][:, it * 128 : (it + 1) * 128]),
                            r(hmrhs[jt][:, 0:502]),
                            start=(jt == 0), stop=(jt == NT - 1))
                    nc.vector.reciprocal(
                        fsc[:, it * 16 + 13 : it * 16 + 14],
                        fsc[:, it * 16 + 14 : it * 16 + 15])
                    # hm (raw) -> SBUF next to pT for the m3 products
                    nc.vector.tensor_copy(ptse[it][:, 2 * H : 3 * H],
                                          hmx[:, 0:H])
                    nc.vector.tensor_copy(ptse[it][:, 3 * H : 4 * H],
                                          hmx[:, 0:H])
                    gout = sml.tile([128, 204], f32, tag="gout")
                    nc.gpsimd.indirect_dma_start(
                        out=gout[:], out_offset=None, in_=GT.ap(),
                        in_offset=bass.IndirectOffsetOnAxis(
                            ap=idx8[it][:, 0:1], axis=0))
                    # m3 products: (hm3u_l * invrs) * [pT,pT,hm,hm]
                    nc.vector.scalar_tensor_tensor(
                        pscr[:],
                        hmx[:, 101:501],
                        fsc[:, it * 16 + 13 : it * 16 + 14],
                        ptse[it][:, 0 : 4 * H],
                        op0=ALU.mult, op1=ALU.mult)
                    nc.vector.reduce_sum(
                        ap3(fsc, it * 16, [[4, 2], [1, 2]]),
                        ap3(pscr, 0, [[100, 4], [1, H]]), axis=AX.X)
                    # m4 products: hmaxT * [p4uT_0 | p4uT_1]
                    nc.vector.tensor_tensor(
                        pscr[:, 0:200], gout[:, 0 : 2 * H],
                        p4u[it][:], op=ALU.mult)
                    nc.vector.reduce_sum(
                        fsc[:, it * 16 + 2 : it * 16 + 4],
                        ap3(pscr, 0, [[100, 2], [1, H]]), axis=AX.X)
                    nc.vector.tensor_copy(
                        fsc[:, it * 16 + 6 : it * 16 + 8],
                        gout[:, 200:202])
                    # d3 *= invrs (second factor -> invrs^2 total)
                    nc.vector.tensor_scalar(
                        fsc[:, it * 16 + 4 : it * 16 + 6],
                        fsc[:, it * 16 + 4 : it * 16 + 6],
                        fsc[:, it * 16 + 13 : it * 16 + 14],
                        None, op0=ALU.mult)

                # ---- phase E: finalize -------------------------------
                nc.scalar.activation(
                    ap3(fsc, 4, [[16, 3], [1, 2]]),
                    ap3(fsc, 4, [[16, 3], [1, 2]]), AF.Sqrt)
                nc.vector.reciprocal(
                    ap3(fsc, 4, [[16, 3], [1, 2]]),
                    ap3(fsc, 4, [[16, 3], [1, 2]]))
                tmp34 = sml.tile([128, 12], f32, tag="tmp34")
                nc.vector.tensor_tensor(
                    tmp34[:], ap3(fsc, 0, [[16, 3], [1, 4]]),
                    ap3(fsc, 4, [[16, 3], [1, 4]]), op=ALU.mult)
                nc.vector.tensor_tensor(
                    ap3(stgt, (4 + di) * 48 + b * 2,
                        [[16, 3], [96, 2], [1, 2]]),
                    tmp34[:], ap3(ipns, 7, [[9, 3], [-6, 2], [1, 2]]),
                    op=ALU.mult)
                tmp12 = sml.tile([128, 12], f32, tag="tmp12")
                nc.vector.tensor_tensor(
                    tmp12[:], ap3(fsc, 11, [[16, 3], [-3, 2], [1, 2]]),
                    ap3(ipns, 5, [[9, 3], [-2, 2], [1, 2]]), op=ALU.mult)
                nc.vector.tensor_tensor(
                    ap3(stgt, di * 48 + b * 2, [[16, 3], [96, 2], [1, 2]]),
                    tmp12[:], ap3(nq4, 0, [[0, 3], [1, 4]]), op=ALU.mult)

        for oi, nm in enumerate(("m1f", "m1b", "m2f", "m2b",
                                 "m3f", "m3b", "m4f", "m4b")):
            t = outs[nm]
            out_ap = AP(t, 0, [[16, 128], [2048, 3], [1, 16]])
            in_ap = ap3(stgt, oi * 48, [[16, 3], [1, 16]])
            nc.sync.dma_start(out_ap, in_ap)

        for p in (ps, stg, sml, big, con):
            p.release()

    nc.compile()
    return nc


def _host_consts(w1, w2, w3f_, w4f_):
    u1 = (w1 * w1).astype(np.float32)
    u2 = (w2 * w2).astype(np.float32)
    u3 = (w3f_ * w3f_).astype(np.float32)
    u4 = (w4f_ * w4f_).astype(np.float32)
    ones = np.ones((H,), np.float32)
    wsb = np.stack([ones, u4[0], u4[1], u2[0], u2[1],
                    u1[0], u1[1], u3[0], u3[1]], axis=1)  # (H, 9)
    return u1, u2, u3, u4, wsb


def _prepare(p_f, p_b, q_f, q_b, w1f, w1b, w2f, w2b, w3f, w3b, w4f, w4b):
    use_fp32_c = os.environ.get("BIMPM_C_FP32", "1") == "1"
    key = ("prog", use_fp32_c)
    if key not in _COMPILED:
        _COMPILED[key] = _build_program(use_fp32_c)
    nc = _COMPILED[key]

    p_f, p_b = np.asarray(p_f), np.asarray(p_b)
    q_f, q_b = np.asarray(q_f), np.asarray(q_b)
    u1f, u2f_, u3, u4, wsbf = _host_consts(
        np.asarray(w1f), np.asarray(w2f), np.asarray(w3f), np.asarray(w4f))
    u1b, u2b_, _, _, wsbb = _host_consts(
        np.asarray(w1b), np.asarray(w2b), np.asarray(w3f), np.asarray(w4f))
    WSB = np.concatenate([wsbf, wsbb], axis=1).astype(np.float32)
    U1C = np.stack([u1f[0], u1f[1], u1b[0], u1b[1]], 1).astype(np.float32)
    U2C = np.stack([u2f_[0], u2f_[1], u2b_[0], u2b_[1]], 1).astype(np.float32)
    U3C = np.stack([u3[0], u3[1], u4[0], u4[1]], 1).astype(np.float32)
    ubc = np.concatenate([u3[0], u3[1], u4[0], u4[1],
                          u3[0], u3[0], u3[1], u3[1]]).astype(np.float32)
    UBC34 = np.ascontiguousarray(np.broadcast_to(ubc, (128, 800)))
    SEL = np.zeros((9, 384), np.float32)
    SEL[2, 0:128] = 1.0    # invq (ones column of WSB)
    SEL[3, 128:256] = 1.0  # invq2_0
    SEL[4, 256:384] = 1.0  # invq2_1
    IDN = np.eye(128, dtype=np.float32)

    in_maps = []
    for c in range(NCORES):
        sl = slice(c * BC, (c + 1) * BC)
        in_maps.append({
            "p_f": np.ascontiguousarray(p_f[:, sl, :]),
            "p_b": np.ascontiguousarray(p_b[:, sl, :]),
            "q_f": np.ascontiguousarray(q_f[:, sl, :]),
            "q_b": np.ascontiguousarray(q_b[:, sl, :]),
            "WSB": WSB, "U1C": U1C, "U2C": U2C, "U3C": U3C, "UBC34": UBC34,
            "SEL": SEL, "IDN": IDN, "SELF": SEL[:, 0:128].copy(),
            "WSBF": WSB,
        })

    return nc, in_maps


def _gather_outputs(results):
    full = []
    for nm in ("m1f", "m1b", "m2f", "m2b", "m3f", "m3b", "m4f", "m4b"):
        full.append(np.concatenate([results[c][nm] for c in range(NCORES)],
                                   axis=1))
    return tuple(full)


def kernel(**inputs):
    from concourse.bass_utils import run_bass_kernel_spmd

    nc, in_maps = _prepare(**inputs)
    res = run_bass_kernel_spmd(nc, in_maps, list(range(NCORES)))
    return _gather_outputs(res.results)


def run_traced(**inputs):
    """Run with NTFF profiling; returns (exec_time_ns, results_obj)."""
    from concourse.bass_utils import run_bass_kernel_spmd

    nc, in_maps = _prepare(**inputs)
    res = run_bass_kernel_spmd(nc, in_maps, list(range(NCORES)), trace=True)
    return res.exec_time_ns, res



# revision 15
# speedup vs baseline: 1.0715x; 1.0715x over previous
"""BiMPM matching-layer kernel for 8 Trainium2 NeuronCores.

Data-parallel over the batch axis: each of the 8 cores gets 8 of the 64
batch elements (full hidden=100 and seq=384 on every core). Weights are
replicated (host-squared into the forms the device needs).

Per (batch, direction): m1 (cosine vs last q timestep), m2 (max pairwise
weighted cosine), m3 (cosine vs attention-mean of q), m4 (cosine vs
argmax-attended q), for l in {0,1}. The backward direction reuses the
*forward* w3/w4 tables (reference bug preserved). Outputs: 8 tensors of
shape (384, 64, 2), order m1f,m1b,m2f,m2b,m3f,m3b,m4f,m4b.

Precision plan: the m3 sign (1/sum of attention) and the m4 argmax are
ill-conditioned (gaps ~1e-4 / ~1e-6), so the p-q̂ attention matmul and
the q norms run in full fp32. Everything else (m2 numerators, hmean,
m4 dot products, p norms) tolerates bf16, which runs the PE at 1
cycle/row with fast weight loads instead of fp32's 4 cycles/row.
"""

import os
import sys

sys.path.insert(0, "/opt/trn_rl_repo")

import numpy as np

H, B, S, L = 100, 64, 384, 2
NCORES = 8
BC = B // NCORES  # 8 batches per core
NT = S // 128  # 3 tiles of 128 along seq
NPAIR = 2 * BC  # 16 (direction, batch) pairs per core

_COMPILED = {}


def _build_program():
    """Builds the single-core SPMD Bass program (same on all 8 cores)."""
    import concourse.bacc as bacc
    import concourse.bass as bass
    import concourse.mybir as mybir
    import concourse.tile as tile
    from concourse.bass_types import AP

    dt = mybir.dt
    f32 = dt.float32
    bf16 = dt.bfloat16
    AF = mybir.ActivationFunctionType
    ALU = mybir.AluOpType
    AX = mybir.AxisListType

    nc = bacc.Bacc("TRN2", target_bir_lowering=False, debug=False)

    ins = {}
    for nm in ("p_f", "p_b", "q_f", "q_b"):
        ins[nm] = nc.dram_tensor(nm, [H, BC, S], f32, kind="ExternalInput")
    WSB16 = nc.dram_tensor("WSB16", [H, 18], bf16, kind="ExternalInput")
    WSBF = nc.dram_tensor("WSBF", [H, 18], f32, kind="ExternalInput")
    U1C = nc.dram_tensor("U1C", [H, 4], f32, kind="ExternalInput")
    U2C = nc.dram_tensor("U2C", [H, 4], f32, kind="ExternalInput")
    UBC16 = nc.dram_tensor("UBC16", [128, 400], bf16, kind="ExternalInput")
    SEL16 = nc.dram_tensor("SEL16", [9, 384], bf16, kind="ExternalInput")
    IDN = nc.dram_tensor("IDN", [128, 128], f32, kind="ExternalInput")
    IDN16 = nc.dram_tensor("IDN16", [128, 128], bf16, kind="ExternalInput")
    outs = {}
    for nm in ("m1f", "m1b", "m2f", "m2b", "m3f", "m3b", "m4f", "m4b"):
        outs[nm] = nc.dram_tensor(nm, [S, BC, L], f32, kind="ExternalOutput")
    # per-pair DRAM scratch for the gather source (bf16 rows)
    GTs = [nc.dram_tensor(f"GT{i}", [S, 104], bf16) for i in range(NPAIR)]

    def ap3(t, off, pattern):
        """AP on tile t: partition dim + explicit free-dim [step,count]s."""
        base = t[:, 0:1]
        part = list(base.ap[0])
        return AP(base.tensor, base.offset + off,
                  [part] + [list(x) for x in pattern])

    with tile.TileContext(nc) as tc:
        con = tc.alloc_tile_pool(name="con", bufs=1)
        big = tc.alloc_tile_pool(name="big", bufs=2)
        sml = tc.alloc_tile_pool(name="sml", bufs=2)
        stg = tc.alloc_tile_pool(name="stg", bufs=1)
        ps = tc.alloc_tile_pool(name="ps", bufs=1, space="PSUM")

        # ---- constants -----------------------------------------------
        idn = con.tile([128, 128], f32, tag="idn")
        nc.sync.dma_start(idn[:], IDN.ap())
        idn16 = con.tile([128, 128], bf16, tag="idn16")
        nc.sync.dma_start(idn16[:], IDN16.ap())
        onesb = con.tile([128, 128], f32, tag="onesb")
        nc.vector.memset(onesb[0:1, :], 1.0)
        ones_row = onesb[0:1, :]
        wsb16_t = con.tile([128, 18], bf16, tag="wsb16")
        nc.sync.dma_start(wsb16_t[0:H, :], WSB16.ap())
        wsb16 = wsb16_t[0:H, :]
        wsbff_t = con.tile([128, 18], f32, tag="wsbff")
        nc.sync.dma_start(wsbff_t[0:H, :], WSBF.ap())
        wsbff = wsbff_t[0:H, :]
        u1c_t = con.tile([128, 4], f32, tag="u1c")
        nc.sync.dma_start(u1c_t[0:H, :], U1C.ap())
        u1c = u1c_t[0:H, :]
        u2c_t = con.tile([128, 4], f32, tag="u2c")
        nc.sync.dma_start(u2c_t[0:H, :], U2C.ap())
        u2c = u2c_t[0:H, :]
        ubc16 = con.tile([128, 400], bf16, tag="ubc16")
        nc.sync.dma_start(ubc16[:], UBC16.ap())
        sel_t = con.tile([128, 384], bf16, tag="sel16")
        nc.sync.dma_start(sel_t[0:9, :], SEL16.ap())
        sel16 = sel_t[0:9, :]
        zeros8 = con.tile([128, 8], f32, tag="zeros8")
        nc.vector.memset(zeros8[:], 0.0)

        # ---- persistent staging --------------------------------------
        # output staging: col = out_idx*48 + it*16 + b*2 + l
        stgt = stg.tile([128, 384], f32, tag="stgt")
        # per-pair scalars: col block pr*48 (layout matches old fsc)
        fscall = stg.tile([128, NPAIR * 48], f32, tag="fscall")
        # per-pair transposed inverse norms (27 cols each)
        ipnall = stg.tile([128, NPAIR * 27], f32, tag="ipnall")
        # per-pair [1/nq_0, 1/nq_1, 1, 1]
        nq4all = stg.tile([128, NPAIR * 4], f32, tag="nq4all")
        nc.vector.memset(ap3(nq4all, 2, [[4, NPAIR], [1, 2]]), 1.0)
        tmp34 = stg.tile([128, NPAIR * 12], f32, tag="tmp34")
        tmp12 = stg.tile([128, NPAIR * 12], f32, tag="tmp12")

        for di, d in enumerate(("f", "b")):
            P_in, Q_in = ins["p_" + d], ins["q_" + d]
            for b in range(BC):
                pr = di * BC + b
                GT = GTs[pr]

                def fscap(off, pattern):
                    return ap3(fscall, pr * 48 + off, pattern)

                fsc = fscall[:, pr * 48 : pr * 48 + 48]

                # ---- phase A: load, norms, scaled q variants ---------
                psbf_t = big.tile([128, S], f32, tag="psbf")
                qsbf_t = big.tile([128, S], f32, tag="qsbf")
                psbf, qsbf = psbf_t[0:H, :], qsbf_t[0:H, :]
                nc.sync.dma_start(psbf[:], P_in.ap()[:, b, :])
                nc.sync.dma_start(qsbf[:], Q_in.ap()[:, b, :])
                psb16_t = big.tile([128, S], bf16, tag="psb16")
                qsb16_t = big.tile([128, S], bf16, tag="qsb16")
                psq16_t = big.tile([128, S], bf16, tag="psq16")
                psb16, qsb16 = psb16_t[0:H, :], qsb16_t[0:H, :]
                psq16 = psq16_t[0:H, :]
                nc.scalar.activation(psb16[:], psbf[:], AF.Copy)
                nc.scalar.activation(qsb16[:], qsbf[:], AF.Copy)
                nc.scalar.activation(psq16[:], psbf[:], AF.Square)
                qsq_t = big.tile([128, S], f32, tag="qsq")
                qsq = qsq_t[0:H, :]
                nc.vector.tensor_tensor(qsq[:], qsbf[:], qsbf[:],
                                        op=ALU.mult)

                pnqn = ps.tile([9, 896], f32, tag="pnqn")
                nc.tensor.matmul(pnqn[:, 0:S],
                                 wsb16[:, di * 9 : di * 9 + 9], psq16[:],
                                 start=True, stop=True)
                nc.tensor.matmul(pnqn[:, 512 : 512 + S],
                                 wsbff[:, di * 9 : di * 9 + 9], qsq[:],
                                 start=True, stop=True)
                ipqn_t = sml.tile([128, 2 * S], f32, tag="ipqn")
                rscr_t = sml.tile([128, 2 * S], f32, tag="rscr")
                ipqn, rscr = ipqn_t[0:9, :], rscr_t[0:9, :]
                from concourse.dve_ops import RECIPROCAL_APPROX_NR
                nc.scalar.activation(ap3(rscr, 0, [[S, 2], [1, S]]),
                                     ap3(pnqn, 0, [[512, 2], [1, S]]),
                                     AF.Sqrt)
                nc.vector.reciprocal_approx_fast(out=ipqn[:], in_=rscr[:])
                nc.vector._custom_dve(RECIPROCAL_APPROX_NR, out=ipqn[:],
                                      in0=rscr[:], in1=ipqn[:], s0=2.0)
                nc.vector._custom_dve(RECIPROCAL_APPROX_NR, out=ipqn[:],
                                      in0=rscr[:], in1=ipqn[:], s0=2.0)
                ipn = ipqn[:, 0:S]
                iqn = ipqn[:, S : 2 * S]
                # rsqrt Newton polish on the q rows: kills the ACT-LUT
                # sqrt error that perturbs argmax columns.
                nra = rscr[:, 0:S]
                nc.vector.tensor_tensor(nra[:], pnqn[:, 512 : 512 + S],
                                        iqn[:], op=ALU.mult)
                nc.vector.tensor_tensor(nra[:], nra[:], iqn[:], op=ALU.mult)
                nc.vector.tensor_scalar(nra[:], nra[:], -0.5, 1.5,
                                        op0=ALU.mult, op1=ALU.add)
                nc.vector.tensor_tensor(iqn[:], iqn[:], nra[:], op=ALU.mult)

                # misc PSUM bank: ipn transposed + m1 scale + iqn4 transp.
                misc = ps.tile([128, 512], f32, tag="misc")
                for it in range(NT):
                    nc.tensor.transpose(misc[:, it * 9 : it * 9 + 9],
                                        ipn[:, it * 128 : (it + 1) * 128],
                                        idn[:9, :9])
                # nq_l^2 = sum(u1_l * qlast^2) -> (1,2) at cols 32:34
                nc.tensor.matmul(misc[0:1, 32:34], qsq[:, S - 1 : S],
                                 u1c[:, di * 2 : di * 2 + 2],
                                 start=True, stop=True)
                nqr = sml.tile([128, 2], f32, tag="nqr")
                nc.scalar.activation(nqr[0:1, :], misc[0:1, 32:34], AF.Sqrt)
                nc.vector.reciprocal(nqr[0:1, :], nqr[0:1, :])
                nc.tensor.matmul(misc[:, 40:42], ones_row[:],
                                 nqr[0:1, :], start=True, stop=True)
                # inverse den4 columns: transpose iqn rows 0:3 per tile
                for it in range(NT):
                    nc.tensor.transpose(
                        misc[:, 64 + it * 3 : 64 + it * 3 + 3],
                        iqn[0:3, it * 128 : (it + 1) * 128],
                        idn[:3, :3])
                nc.vector.tensor_copy(ipnall[:, pr * 27 : pr * 27 + 27],
                                      misc[:, 0:27])
                nc.vector.tensor_copy(nq4all[:, pr * 4 : pr * 4 + 2],
                                      misc[:, 40:42])
                iq4t = sml.tile([128, 9], f32, tag="iq4t")
                nc.vector.tensor_copy(iq4t[:], misc[:, 64:73])

                # q variants: qn = q/|q| (fp32), q2n = q*u2_l/|w2_l q| (bf16)
                qn_t = big.tile([128, S + 2], f32, tag="qn")
                qn = qn_t[0:H, :]
                q2a_t = big.tile([128, 2 * S], bf16, tag="q2a")
                q2n_t = big.tile([128, 2 * S], bf16, tag="q2n")
                q2a, q2n = q2a_t[0:H, :], q2n_t[0:H, :]
                nc.scalar.activation(q2a[:, 0:S], qsbf[:], AF.Copy,
                                     scale=u2c[:, di * 2 : di * 2 + 1])
                nc.scalar.activation(q2a[:, S : 2 * S], qsbf[:], AF.Copy,
                                     scale=u2c[:, di * 2 + 1 : di * 2 + 2])
                iqr16_t = sml.tile([128, S], bf16, tag="iqr16")
                iqr16 = iqr16_t[0:5, :]
                nc.vector.tensor_copy(iqr16[:], iqn[0:5, :])
                bcq = sml.tile([128, S], f32, tag="bcq")
                nc.gpsimd.partition_broadcast(bcq[0:H, :], iqn[0:1, :])
                nc.vector.tensor_tensor(qn[:, 0:S], qsbf[:], bcq[0:H, :],
                                        op=ALU.mult)
                nc.vector.tensor_tensor(
                    qn[:, S : S + 2],
                    qsbf[:, S - 1 : S].to_broadcast([H, 2]),
                    u1c[:, di * 2 : di * 2 + 2],
                    op=ALU.mult)
                for k in (1, 2):
                    bc = ps.tile([128, 512], f32, tag="mega")
                    nc.tensor.matmul(
                        bc[0:H, 0:S],
                        sel16[0:5, k * 128 : k * 128 + H],
                        iqr16[:], start=True, stop=True)
                    off = (k - 1) * S
                    nc.vector.tensor_tensor(q2n[:, off : off + S],
                                            q2a[:, off : off + S],
                                            bc[0:H, 0:S], op=ALU.mult)

                # ---- phase B: i-layout matmuls + fused maxes ---------
                idx8 = [sml.tile([128, 8], dt.uint32, tag=f"idx{it}",
                                 name=f"idx{it}")
                        for it in range(NT)]
                att16 = big.tile([128, S], bf16, tag="att16")
                cts16 = big.tile([128, NT * S], bf16, tag="cts16")
                for it in range(NT):
                    mega = ps.tile([128, 1536], f32, tag="mega")
                    lhs16 = psb16[:, it * 128 : (it + 1) * 128]
                    nc.tensor.matmul(mega[:, 0:S], lhs16,
                                     q2n[:, 0:S], start=True, stop=True)
                    nc.tensor.matmul(mega[:, 512 : 512 + S], lhs16,
                                     q2n[:, S : 2 * S],
                                     start=True, stop=True)
                    nc.tensor.matmul(
                        mega[:, 1024 : 1024 + S + 2],
                        psbf[:, it * 128 : (it + 1) * 128],
                        qn[:], start=True, stop=True)
                    nc.vector.reduce_max(
                        fsc[:, it * 16 + 8 : it * 16 + 11],
                        ap3(mega, 0, [[512, 3], [1, S]]), axis=AX.X)
                    mx8 = sml.tile([128, 8], f32, tag="mx8")
                    nc.vector.tensor_scalar(
                        mx8[:], zeros8[:],
                        fsc[:, it * 16 + 10 : it * 16 + 11], None,
                        op0=ALU.add)
                    nc.vector.max_index(
                        idx8[it][:], mx8[:],
                        mega[:, 1024 : 1024 + S])
                    nc.vector.tensor_copy(
                        fsc[:, it * 16 + 11 : it * 16 + 13],
                        mega[:, 1024 + S : 1024 + S + 2])
                    # row copy in bf16 (reused as hmean lhsT) + row sum
                    # for the m3 attention denominator -- on ScalarE.
                    nc.scalar.activation(
                        att16[:], mega[:, 1024 : 1024 + S], AF.Copy,
                        accum_out=fsc[:, it * 16 + 14 : it * 16 + 15])
                    # transpose p.q-hat into j-major layout for hmean
                    ctt = ps.tile([128, 384], bf16, tag="ctt")
                    for jt in range(NT):
                        nc.tensor.transpose(
                            ctt[:, jt * 128 : (jt + 1) * 128],
                            att16[:, jt * 128 : (jt + 1) * 128],
                            idn16[:, :])
                    nc.vector.tensor_copy(
                        ap3(cts16, it * 128, [[S, NT], [1, 128]]),
                        ctt[:])

                # ---- phase C: T-layout rhs packs ---------------------
                hmrhs = [sml.tile([128, 304], bf16, tag=f"hmrhs{jt}",
                                  name=f"hmrhs{jt}")
                         for jt in range(NT)]
                ptse = [sml.tile([128, 200], bf16, tag=f"ptse{jt}",
                                 name=f"ptse{jt}")
                        for jt in range(NT)]
                p4u = [sml.tile([128, 200], bf16, tag=f"p4u{jt}",
                                name=f"p4u{jt}")
                       for jt in range(NT)]
                for jt in range(NT):
                    tqp = ps.tile([128, 224], bf16, tag="misc")
                    nc.tensor.transpose(tqp[:, 0:H],
                                        qsb16[:, jt * 128 : (jt + 1) * 128],
                                        idn16[:H, :H])
                    nc.tensor.transpose(tqp[:, 112 : 112 + H],
                                        psb16[:, jt * 128 : (jt + 1) * 128],
                                        idn16[:H, :H])
                    nc.vector.tensor_copy(hmrhs[jt][:, 0:H], tqp[:, 0:H])
                    # q3uT_l = qT * u3_l for l=0,1 (read qT twice)
                    nc.vector.tensor_tensor(
                        hmrhs[jt][:, H : 3 * H],
                        ap3(hmrhs[jt], 0, [[0, 2], [1, H]]),
                        ubc16[:, 0 : 2 * H], op=ALU.mult)
                    gsrc = sml.tile([128, 104], bf16, tag="gsrc")
                    nc.vector.tensor_copy(gsrc[:, 0:H], tqp[:, 0:H])
                    nc.vector.tensor_copy(gsrc[:, H : H + 2],
                                          iq4t[:, jt * 3 + 1 : jt * 3 + 3])
                    nc.vector.memset(gsrc[:, H + 2 : H + 4], 0.0)
                    nc.sync.dma_start(
                        GT.ap()[jt * 128 : (jt + 1) * 128, :], gsrc[:])
                    nc.vector.tensor_copy(ptse[jt][:, 0:H],
                                          tqp[:, 112 : 112 + H])
                    # p4uT pack = [pT*u4_0 | pT*u4_1]
                    nc.vector.tensor_tensor(
                        p4u[jt][:],
                        ap3(ptse[jt], 0, [[0, 2], [1, H]]),
                        ubc16[:, 2 * H : 4 * H], op=ALU.mult)

                # ---- phase D: hm matmuls, gather, dot products -------
                pscr = sml.tile([128, 400], f32, tag="pscr")
                for it in range(NT):
                    hmx = ps.tile([128, 512], f32, tag="hmx")
                    for jt in range(NT):
                        nc.tensor.matmul(
                            hmx[:, 0:300],
                            cts16[:, jt * S + it * 128
                                  : jt * S + (it + 1) * 128],
                            hmrhs[jt][:, 0:300],
                            start=(jt == 0), stop=(jt == NT - 1))
                    nc.vector.reciprocal(
                        fsc[:, it * 16 + 13 : it * 16 + 14],
                        fsc[:, it * 16 + 14 : it * 16 + 15])
                    # hm (raw) -> next to pT for the m3 d3 products
                    nc.vector.tensor_copy(ptse[it][:, H : 2 * H],
                                          hmx[:, 0:H])
                    gout = sml.tile([128, 104], bf16, tag="gout")
                    nc.gpsimd.indirect_dma_start(
                        out=gout[:], out_offset=None, in_=GT.ap(),
                        in_offset=bass.IndirectOffsetOnAxis(
                            ap=idx8[it][:, 0:1], axis=0))
                    # m3 products: (hm3u_l * invrs) * [pT,pT] then [hm,hm]
                    nc.vector.scalar_tensor_tensor(
                        pscr[:, 0 : 2 * H],
                        hmx[:, H : 3 * H],
                        fsc[:, it * 16 + 13 : it * 16 + 14],
                        ap3(ptse[it], 0, [[0, 2], [1, H]]),
                        op0=ALU.mult, op1=ALU.mult)
                    nc.vector.scalar_tensor_tensor(
                        pscr[:, 2 * H : 4 * H],
                        hmx[:, H : 3 * H],
                        fsc[:, it * 16 + 13 : it * 16 + 14],
                        ap3(ptse[it], H, [[0, 2], [1, H]]),
                        op0=ALU.mult, op1=ALU.mult)
                    nc.vector.reduce_sum(
                        fscap(it * 16, [[4, 2], [1, 2]]),
                        ap3(pscr, 0, [[100, 4], [1, H]]), axis=AX.X)
                    # m4 products: hmaxT (read twice) * [p4uT_0 | p4uT_1]
                    nc.vector.tensor_tensor(
                        pscr[:, 0:200],
                        ap3(gout, 0, [[0, 2], [1, H]]),
                        p4u[it][:], op=ALU.mult)
                    nc.vector.reduce_sum(
                        fsc[:, it * 16 + 2 : it * 16 + 4],
                        ap3(pscr, 0, [[100, 2], [1, H]]), axis=AX.X)
                    nc.vector.tensor_copy(
                        fsc[:, it * 16 + 6 : it * 16 + 8],
                        gout[:, H : H + 2])
                    # d3 *= invrs (second factor -> invrs^2 total)
                    nc.vector.tensor_scalar(
                        fsc[:, it * 16 + 4 : it * 16 + 6],
                        fsc[:, it * 16 + 4 : it * 16 + 6],
                        fsc[:, it * 16 + 13 : it * 16 + 14],
                        None, op0=ALU.mult)

        # ---- phase E: finalize all pairs at once ---------------------
        # 1/sqrt over the m3/m4 denominators (cols +4..+8 per it block)
        den = ap3(fscall, 4, [[48, NPAIR], [16, 3], [1, 2]])
        nc.scalar.activation(den, den, AF.Sqrt)
        nc.vector.reciprocal(den, den)
        # m3/m4: tmp34 = num * invden, then * transposed 1/|w p| norms
        nc.vector.tensor_tensor(
            tmp34[:], ap3(fscall, 0, [[48, NPAIR], [16, 3], [1, 4]]),
            ap3(fscall, 4, [[48, NPAIR], [16, 3], [1, 4]]), op=ALU.mult)
        for di in range(2):
            for o in range(2):  # o=0: m3, o=1: m4
                nc.vector.tensor_tensor(
                    ap3(stgt, (4 + di) * 48 + 96 * o,
                        [[2, BC], [16, 3], [1, 2]]),
                    ap3(tmp34, di * BC * 12 + o * 2,
                        [[12, BC], [4, 3], [1, 2]]),
                    ap3(ipnall, di * BC * 27 + (7 if o == 0 else 1),
                        [[27, BC], [9, 3], [1, 2]]),
                    op=ALU.mult)
        # m1/m2: tmp12 = num * 1/|w p|, then m1 *= 1/nq
        for di in range(2):
            for o in range(2):  # o=0: m1, o=1: m2
                nc.vector.tensor_tensor(
                    ap3(tmp12, di * BC * 12 + o * 2,
                        [[12, BC], [4, 3], [1, 2]]),
                    ap3(fscall, di * BC * 48 + (11 if o == 0 else 8),
                        [[48, BC], [16, 3], [1, 2]]),
                    ap3(ipnall, di * BC * 27 + (5 if o == 0 else 3),
                        [[27, BC], [9, 3], [1, 2]]),
                    op=ALU.mult)
                nc.vector.tensor_tensor(
                    ap3(stgt, di * 48 + 96 * o, [[2, BC], [16, 3], [1, 2]]),
                    ap3(tmp12, di * BC * 12 + o * 2,
                        [[12, BC], [4, 3], [1, 2]]),
                    ap3(nq4all, di * BC * 4 + (0 if o == 0 else 2),
                        [[4, BC], [0, 3], [1, 2]]),
                    op=ALU.mult)

        for oi, nm in enumerate(("m1f", "m1b", "m2f", "m2b",
                                 "m3f", "m3b", "m4f", "m4b")):
            t = outs[nm]
            out_ap = AP(t, 0, [[16, 128], [2048, 3], [1, 16]])
            in_ap = ap3(stgt, oi * 48, [[16, 3], [1, 16]])
            nc.sync.dma_start(out_ap, in_ap)

        for p in (ps, stg, sml, big, con):
            p.release()

    nc.compile()
    return nc


def _host_consts(w1, w2, w3f_, w4f_):
    u1 = (w1 * w1).astype(np.float32)
    u2 = (w2 * w2).astype(np.float32)
    u3 = (w3f_ * w3f_).astype(np.float32)
    u4 = (w4f_ * w4f_).astype(np.float32)
    ones = np.ones((H,), np.float32)
    wsb = np.stack([ones, u4[0], u4[1], u2[0], u2[1],
                    u1[0], u1[1], u3[0], u3[1]], axis=1)  # (H, 9)
    return u1, u2, u3, u4, wsb


def _prepare(p_f, p_b, q_f, q_b, w1f, w1b, w2f, w2b, w3f, w3b, w4f, w4b):
    import ml_dtypes

    bfd = ml_dtypes.bfloat16
    if "prog" not in _COMPILED:
        _COMPILED["prog"] = _build_program()
    nc = _COMPILED["prog"]

    p_f, p_b = np.asarray(p_f), np.asarray(p_b)
    q_f, q_b = np.asarray(q_f), np.asarray(q_b)
    u1f, u2f_, u3, u4, wsbf = _host_consts(
        np.asarray(w1f), np.asarray(w2f), np.asarray(w3f), np.asarray(w4f))
    u1b, u2b_, _, _, wsbb = _host_consts(
        np.asarray(w1b), np.asarray(w2b), np.asarray(w3f), np.asarray(w4f))
    WSB = np.concatenate([wsbf, wsbb], axis=1).astype(np.float32)
    U1C = np.stack([u1f[0], u1f[1], u1b[0], u1b[1]], 1).astype(np.float32)
    U2C = np.stack([u2f_[0], u2f_[1], u2b_[0], u2b_[1]], 1).astype(np.float32)
    ubc = np.concatenate([u3[0], u3[1], u4[0], u4[1]]).astype(np.float32)
    UBC16 = np.ascontiguousarray(
        np.broadcast_to(ubc, (128, 400))).astype(bfd)
    SEL16 = np.zeros((9, 384), np.float32)
    SEL16[3, 128:256] = 1.0  # invq2_0
    SEL16[4, 256:384] = 1.0  # invq2_1
    IDN = np.eye(128, dtype=np.float32)

    in_maps = []
    for c in range(NCORES):
        sl = slice(c * BC, (c + 1) * BC)
        in_maps.append({
            "p_f": np.ascontiguousarray(p_f[:, sl, :]),
            "p_b": np.ascontiguousarray(p_b[:, sl, :]),
            "q_f": np.ascontiguousarray(q_f[:, sl, :]),
            "q_b": np.ascontiguousarray(q_b[:, sl, :]),
            "WSB16": WSB.astype(bfd), "WSBF": WSB,
            "U1C": U1C, "U2C": U2C, "UBC16": UBC16,
            "SEL16": SEL16.astype(bfd),
            "IDN": IDN, "IDN16": IDN.astype(bfd),
        })

    return nc, in_maps


def _gather_outputs(results):
    full = []
    for nm in ("m1f", "m1b", "m2f", "m2b", "m3f", "m3b", "m4f", "m4b"):
        full.append(np.concatenate([results[c][nm] for c in range(NCORES)],
                                   axis=1))
    return tuple(full)


def kernel(**inputs):
    from concourse.bass_utils import run_bass_kernel_spmd

    nc, in_maps = _prepare(**inputs)
    res = run_bass_kernel_spmd(nc, in_maps, list(range(NCORES)))
    return _gather_outputs(res.results)


def run_traced(**inputs):
    """Run with NTFF profiling; returns (exec_time_ns, results_obj)."""
    from concourse.bass_utils import run_bass_kernel_spmd

    nc, in_maps = _prepare(**inputs)
    res = run_bass_kernel_spmd(nc, in_maps, list(range(NCORES)), trace=True)
    return res.exec_time_ns, res


# revision 18
# speedup vs baseline: 1.2655x; 1.1811x over previous
"""BiMPM matching-layer kernel for 8 Trainium2 NeuronCores.

Data-parallel over the batch axis: each of the 8 cores gets 8 of the 64
batch elements (full hidden=100 and seq=384 on every core). Weights are
replicated (host-squared into the forms the device needs).

Per (batch, direction): m1 (cosine vs last q timestep), m2 (max pairwise
weighted cosine), m3 (cosine vs attention-mean of q), m4 (cosine vs
argmax-attended q), for l in {0,1}. The backward direction reuses the
*forward* w3/w4 tables (reference bug preserved). Outputs: 8 tensors of
shape (384, 64, 2), order m1f,m1b,m2f,m2b,m3f,m3b,m4f,m4b.

Precision plan: the m3 sign (1/sum of attention) and the m4 argmax are
ill-conditioned (gaps ~1e-4 / ~1e-6), so the p-q̂ attention matmul and
the q norms run in full fp32. Everything else (m2 numerators, hmean,
m4 dot products, p norms) tolerates bf16, which runs the PE at 1
cycle/row with fast weight loads instead of fp32's 4 cycles/row.
"""

import os
import sys

sys.path.insert(0, "/opt/trn_rl_repo")

import numpy as np

H, B, S, L = 100, 64, 384, 2
NCORES = 8
BC = B // NCORES  # 8 batches per core
NT = S // 128  # 3 tiles of 128 along seq
NPAIR = 2 * BC  # 16 (direction, batch) pairs per core

_COMPILED = {}


def _build_program():
    """Builds the single-core SPMD Bass program (same on all 8 cores)."""
    import concourse.bacc as bacc
    import concourse.bass as bass
    import concourse.mybir as mybir
    import concourse.tile as tile
    from concourse.bass_types import AP

    dt = mybir.dt
    f32 = dt.float32
    bf16 = dt.bfloat16
    AF = mybir.ActivationFunctionType
    ALU = mybir.AluOpType
    AX = mybir.AxisListType

    nc = bacc.Bacc("TRN2", target_bir_lowering=False, debug=False)

    ins = {}
    for nm in ("p_f", "p_b", "q_f", "q_b"):
        ins[nm] = nc.dram_tensor(nm, [H, BC, S], f32, kind="ExternalInput")
    WSB16 = nc.dram_tensor("WSB16", [H, 18], bf16, kind="ExternalInput")
    WSBF = nc.dram_tensor("WSBF", [H, 18], f32, kind="ExternalInput")
    U1C = nc.dram_tensor("U1C", [H, 4], f32, kind="ExternalInput")
    U2C = nc.dram_tensor("U2C", [H, 4], f32, kind="ExternalInput")
    UBC16 = nc.dram_tensor("UBC16", [128, 400], bf16, kind="ExternalInput")
    SEL16 = nc.dram_tensor("SEL16", [9, 384], bf16, kind="ExternalInput")
    IDN = nc.dram_tensor("IDN", [128, 128], f32, kind="ExternalInput")
    IDN16 = nc.dram_tensor("IDN16", [128, 128], bf16, kind="ExternalInput")
    outs = {}
    for nm in ("m1f", "m1b", "m2f", "m2b", "m3f", "m3b", "m4f", "m4b"):
        outs[nm] = nc.dram_tensor(nm, [S, BC, L], f32, kind="ExternalOutput")
    # per-pair DRAM scratch for the gather source (bf16 rows)
    GTs = [nc.dram_tensor(f"GT{i}", [S, 104], bf16) for i in range(NPAIR)]

    def ap3(t, off, pattern):
        """AP on tile t: partition dim + explicit free-dim [step,count]s."""
        base = t[:, 0:1]
        part = list(base.ap[0])
        return AP(base.tensor, base.offset + off,
                  [part] + [list(x) for x in pattern])

    with tile.TileContext(nc) as tc:
        con = tc.alloc_tile_pool(name="con", bufs=1)
        big = tc.alloc_tile_pool(name="big", bufs=2)
        sml = tc.alloc_tile_pool(name="sml", bufs=2)
        stg = tc.alloc_tile_pool(name="stg", bufs=1)
        ps = tc.alloc_tile_pool(name="ps", bufs=1, space="PSUM")

        # ---- constants -----------------------------------------------
        idn = con.tile([128, 128], f32, tag="idn")
        nc.sync.dma_start(idn[:], IDN.ap())
        idn16 = con.tile([128, 128], bf16, tag="idn16")
        nc.sync.dma_start(idn16[:], IDN16.ap())
        onesb = con.tile([128, 128], f32, tag="onesb")
        nc.vector.memset(onesb[0:1, :], 1.0)
        ones_row = onesb[0:1, :]
        wsb16_t = con.tile([128, 18], bf16, tag="wsb16")
        nc.sync.dma_start(wsb16_t[0:H, :], WSB16.ap())
        wsb16 = wsb16_t[0:H, :]
        wsbff_t = con.tile([128, 18], f32, tag="wsbff")
        nc.sync.dma_start(wsbff_t[0:H, :], WSBF.ap())
        wsbff = wsbff_t[0:H, :]
        u1c_t = con.tile([128, 4], f32, tag="u1c")
        nc.sync.dma_start(u1c_t[0:H, :], U1C.ap())
        u1c = u1c_t[0:H, :]
        u2c_t = con.tile([128, 4], f32, tag="u2c")
        nc.sync.dma_start(u2c_t[0:H, :], U2C.ap())
        u2c = u2c_t[0:H, :]
        ubc16 = con.tile([128, 400], bf16, tag="ubc16")
        nc.sync.dma_start(ubc16[:], UBC16.ap())
        sel_t = con.tile([128, 384], bf16, tag="sel16")
        nc.sync.dma_start(sel_t[0:9, :], SEL16.ap())
        sel16 = sel_t[0:9, :]
        zeros8 = con.tile([128, 8], f32, tag="zeros8")
        nc.vector.memset(zeros8[:], 0.0)

        # ---- persistent staging --------------------------------------
        # output staging: col = out_idx*48 + it*16 + b*2 + l
        stgt = stg.tile([128, 384], f32, tag="stgt")
        # per-pair scalars: col block pr*48 (layout matches old fsc)
        fscall = stg.tile([128, NPAIR * 48], f32, tag="fscall")
        # per-pair transposed inverse norms (27 cols each)
        ipnall = stg.tile([128, NPAIR * 27], f32, tag="ipnall")
        # per-pair [1/nq_0, 1/nq_1, 1, 1]
        nq4all = stg.tile([128, NPAIR * 4], f32, tag="nq4all")
        nc.vector.memset(ap3(nq4all, 2, [[4, NPAIR], [1, 2]]), 1.0)
        tmp34 = stg.tile([128, NPAIR * 12], f32, tag="tmp34")
        tmp12 = stg.tile([128, NPAIR * 12], f32, tag="tmp12")

        for di, d in enumerate(("f", "b")):
            P_in, Q_in = ins["p_" + d], ins["q_" + d]
            for b in range(BC):
                pr = di * BC + b
                GT = GTs[pr]

                def fscap(off, pattern):
                    return ap3(fscall, pr * 48 + off, pattern)

                fsc = fscall[:, pr * 48 : pr * 48 + 48]

                # ---- phase A: load, norms, scaled q variants ---------
                psbf_t = big.tile([128, S], f32, tag="psbf")
                qsbf_t = big.tile([128, S], f32, tag="qsbf")
                psbf, qsbf = psbf_t[0:H, :], qsbf_t[0:H, :]
                nc.sync.dma_start(psbf[:], P_in.ap()[:, b, :])
                nc.sync.dma_start(qsbf[:], Q_in.ap()[:, b, :])
                psb16_t = big.tile([128, S], bf16, tag="psb16")
                qsb16_t = big.tile([128, S], bf16, tag="qsb16")
                psq16_t = big.tile([128, S], bf16, tag="psq16")
                psb16, qsb16 = psb16_t[0:H, :], qsb16_t[0:H, :]
                psq16 = psq16_t[0:H, :]
                nc.scalar.activation(psb16[:], psbf[:], AF.Copy)
                nc.scalar.activation(qsb16[:], qsbf[:], AF.Copy)
                nc.scalar.activation(psq16[:], psbf[:], AF.Square)
                qsq_t = big.tile([128, S], f32, tag="qsq")
                qsq = qsq_t[0:H, :]
                nc.vector.tensor_tensor(qsq[:], qsbf[:], qsbf[:],
                                        op=ALU.mult)

                pnqn = ps.tile([9, 896], f32, tag="pnqn")
                nc.tensor.matmul(pnqn[:, 0:S],
                                 wsb16[:, di * 9 : di * 9 + 9], psq16[:],
                                 start=True, stop=True)
                nc.tensor.matmul(pnqn[:, 512 : 512 + S],
                                 wsbff[:, di * 9 : di * 9 + 9], qsq[:],
                                 start=True, stop=True)
                ipqn_t = sml.tile([128, 2 * S], f32, tag="ipqn")
                rscr_t = sml.tile([128, 2 * S], f32, tag="rscr")
                ipqn, rscr = ipqn_t[0:9, :], rscr_t[0:9, :]
                from concourse.dve_ops import RECIPROCAL_APPROX_NR
                nc.scalar.activation(ap3(rscr, 0, [[S, 2], [1, S]]),
                                     ap3(pnqn, 0, [[512, 2], [1, S]]),
                                     AF.Sqrt)
                nc.vector.reciprocal_approx_fast(out=ipqn[:], in_=rscr[:])
                nc.vector._custom_dve(RECIPROCAL_APPROX_NR, out=ipqn[:],
                                      in0=rscr[:], in1=ipqn[:], s0=2.0)
                nc.vector._custom_dve(RECIPROCAL_APPROX_NR, out=ipqn[:],
                                      in0=rscr[:], in1=ipqn[:], s0=2.0)
                ipn = ipqn[:, 0:S]
                iqn = ipqn[:, S : 2 * S]
                # rsqrt Newton polish on the q rows: kills the ACT-LUT
                # sqrt error that perturbs argmax columns.
                nra = rscr[:, 0:S]
                nc.vector.tensor_tensor(nra[:], pnqn[:, 512 : 512 + S],
                                        iqn[:], op=ALU.mult)
                nc.vector.tensor_tensor(nra[:], nra[:], iqn[:], op=ALU.mult)
                nc.vector.tensor_scalar(nra[:], nra[:], -0.5, 1.5,
                                        op0=ALU.mult, op1=ALU.add)
                nc.vector.tensor_tensor(iqn[:], iqn[:], nra[:], op=ALU.mult)

                # misc PSUM bank: ipn transposed + m1 scale + iqn4 transp.
                misc = ps.tile([128, 512], f32, tag="misc")
                for it in range(NT):
                    nc.tensor.transpose(misc[:, it * 9 : it * 9 + 9],
                                        ipn[:, it * 128 : (it + 1) * 128],
                                        idn[:9, :9])
                # nq_l^2 = sum(u1_l * qlast^2) -> (1,2) at cols 32:34
                nc.tensor.matmul(misc[0:1, 32:34], qsq[:, S - 1 : S],
                                 u1c[:, di * 2 : di * 2 + 2],
                                 start=True, stop=True)
                nqr = sml.tile([128, 2], f32, tag="nqr")
                nc.scalar.activation(nqr[0:1, :], misc[0:1, 32:34], AF.Sqrt)
                nc.vector.reciprocal(nqr[0:1, :], nqr[0:1, :])
                nc.tensor.matmul(misc[:, 40:42], ones_row[:],
                                 nqr[0:1, :], start=True, stop=True)
                # inverse den4 columns: transpose iqn rows 0:3 per tile
                for it in range(NT):
                    nc.tensor.transpose(
                        misc[:, 64 + it * 3 : 64 + it * 3 + 3],
                        iqn[0:3, it * 128 : (it + 1) * 128],
                        idn[:3, :3])
                nc.vector.tensor_copy(ipnall[:, pr * 27 : pr * 27 + 27],
                                      misc[:, 0:27])
                nc.vector.tensor_copy(nq4all[:, pr * 4 : pr * 4 + 2],
                                      misc[:, 40:42])
                iq4t = sml.tile([128, 9], f32, tag="iq4t")
                nc.vector.tensor_copy(iq4t[:], misc[:, 64:73])

                # q variants: qn = q/|q| (fp32), q2n = q*u2_l/|w2_l q| (bf16)
                qn_t = big.tile([128, S + 2], f32, tag="qn")
                qn = qn_t[0:H, :]
                q2a_t = big.tile([128, 2 * S], bf16, tag="q2a")
                q2n_t = big.tile([128, 2 * S], bf16, tag="q2n")
                q2a, q2n = q2a_t[0:H, :], q2n_t[0:H, :]
                nc.scalar.activation(q2a[:, 0:S], qsbf[:], AF.Copy,
                                     scale=u2c[:, di * 2 : di * 2 + 1])
                nc.scalar.activation(q2a[:, S : 2 * S], qsbf[:], AF.Copy,
                                     scale=u2c[:, di * 2 + 1 : di * 2 + 2])
                iqr16_t = sml.tile([128, S], bf16, tag="iqr16")
                iqr16 = iqr16_t[0:5, :]
                nc.vector.tensor_copy(iqr16[:], iqn[0:5, :])
                bcq = sml.tile([128, S], f32, tag="bcq")
                nc.gpsimd.partition_broadcast(bcq[0:H, :], iqn[0:1, :])
                nc.vector.tensor_tensor(qn[:, 0:S], qsbf[:], bcq[0:H, :],
                                        op=ALU.mult)
                nc.vector.tensor_tensor(
                    qn[:, S : S + 2],
                    qsbf[:, S - 1 : S].to_broadcast([H, 2]),
                    u1c[:, di * 2 : di * 2 + 2],
                    op=ALU.mult)
                for k in (1, 2):
                    bc = ps.tile([128, 384], f32, tag="ctt")
                    nc.tensor.matmul(
                        bc[0:H, 0:S],
                        sel16[0:5, k * 128 : k * 128 + H],
                        iqr16[:], start=True, stop=True)
                    off = (k - 1) * S
                    nc.vector.tensor_tensor(q2n[:, off : off + S],
                                            q2a[:, off : off + S],
                                            bc[0:H, 0:S], op=ALU.mult)

                # ---- phase B: i-layout matmuls + fused maxes ---------
                idx8 = [sml.tile([128, 8], dt.uint32, tag=f"idx{it}",
                                 name=f"idx{it}")
                        for it in range(NT)]
                att16 = big.tile([128, S], bf16, tag="att16")
                cts16 = big.tile([128, NT * S], bf16, tag="cts16")
                for it in range(NT):
                    mega = ps.tile([128, 1536], f32, tag="mega")
                    lhs16 = psb16[:, it * 128 : (it + 1) * 128]
                    nc.tensor.matmul(mega[:, 0:S], lhs16,
                                     q2n[:, 0:S], start=True, stop=True)
                    nc.tensor.matmul(mega[:, 512 : 512 + S], lhs16,
                                     q2n[:, S : 2 * S],
                                     start=True, stop=True)
                    nc.tensor.matmul(
                        mega[:, 1024 : 1024 + S + 2],
                        psbf[:, it * 128 : (it + 1) * 128],
                        qn[:], start=True, stop=True)
                    nc.vector.reduce_max(
                        fsc[:, it * 16 + 8 : it * 16 + 11],
                        ap3(mega, 0, [[512, 3], [1, S]]), axis=AX.X)
                    mx8 = sml.tile([128, 8], f32, tag="mx8")
                    nc.vector.tensor_scalar(
                        mx8[:], zeros8[:],
                        fsc[:, it * 16 + 10 : it * 16 + 11], None,
                        op0=ALU.add)
                    nc.vector.max_index(
                        idx8[it][:], mx8[:],
                        mega[:, 1024 : 1024 + S])
                    nc.vector.tensor_copy(
                        fsc[:, it * 16 + 11 : it * 16 + 13],
                        mega[:, 1024 + S : 1024 + S + 2])
                    # row copy in bf16 (reused as hmean lhsT) + row sum
                    # for the m3 attention denominator -- on ScalarE.
                    nc.scalar.activation(
                        att16[:], mega[:, 1024 : 1024 + S], AF.Copy,
                        accum_out=fsc[:, it * 16 + 14 : it * 16 + 15])
                    # transpose p.q-hat into j-major layout for hmean
                    ctt = ps.tile([128, 384], bf16, tag="ctt")
                    for jt in range(NT):
                        nc.tensor.transpose(
                            ctt[:, jt * 128 : (jt + 1) * 128],
                            att16[:, jt * 128 : (jt + 1) * 128],
                            idn16[:, :])
                    nc.vector.tensor_copy(
                        ap3(cts16, it * 128, [[S, NT], [1, 128]]),
                        ctt[:])

                # ---- phase C: T-layout rhs packs (batched over jt) ---
                hmrhs = sml.tile([128, 3 * 304], bf16, tag="hmrhs")
                ptsa = sml.tile([128, 600], bf16, tag="ptsa")
                p4ua = sml.tile([128, 600], bf16, tag="p4ua")
                gsrc = sml.tile([128, 312], bf16, tag="gsrc")
                tqp = ps.tile([128, 672], bf16, tag="misc")
                for jt in range(NT):
                    nc.tensor.transpose(tqp[:, jt * 224 : jt * 224 + H],
                                        qsb16[:, jt * 128 : (jt + 1) * 128],
                                        idn16[:H, :H])
                    nc.tensor.transpose(
                        tqp[:, jt * 224 + 112 : jt * 224 + 112 + H],
                        psb16[:, jt * 128 : (jt + 1) * 128],
                        idn16[:H, :H])
                nc.vector.tensor_copy(ap3(hmrhs, 0, [[304, 3], [1, H]]),
                                      ap3(tqp, 0, [[224, 3], [1, H]]))
                # q3uT_l = qT * u3_l for l=0,1 (read qT twice per jt)
                nc.vector.tensor_tensor(
                    ap3(hmrhs, H, [[304, 3], [1, 2 * H]]),
                    ap3(hmrhs, 0, [[304, 3], [0, 2], [1, H]]),
                    ap3(ubc16, 0, [[0, 3], [1, 2 * H]]), op=ALU.mult)
                nc.vector.tensor_copy(ap3(gsrc, 0, [[104, 3], [1, H]]),
                                      ap3(tqp, 0, [[224, 3], [1, H]]))
                nc.vector.tensor_copy(ap3(gsrc, H, [[104, 3], [1, 2]]),
                                      ap3(iq4t, 1, [[3, 3], [1, 2]]))
                nc.vector.memset(ap3(gsrc, H + 2, [[104, 3], [1, 2]]), 0.0)
                nc.sync.dma_start(
                    AP(GT, 0, [[104, 128], [128 * 104, 3], [1, 104]]),
                    ap3(gsrc, 0, [[104, 3], [1, 104]]))
                nc.vector.tensor_copy(ap3(ptsa, 0, [[200, 3], [1, H]]),
                                      ap3(tqp, 112, [[224, 3], [1, H]]))
                # p4uT pack = [pT*u4_0 | pT*u4_1] per it
                nc.vector.tensor_tensor(
                    p4ua[:],
                    ap3(ptsa, 0, [[200, 3], [0, 2], [1, H]]),
                    ap3(ubc16, 2 * H, [[0, 3], [1, 2 * H]]), op=ALU.mult)

                # ---- phase D: hm matmuls, gather, dot products -------
                # m3 = sign(sum_att) * n3_raw / sqrt(d3_raw): the 1/sum
                # magnitude cancels in the cosine, only its sign matters
                # (applied in phase E), so no reciprocal is needed here.
                pscr = sml.tile([128, 1200], f32, tag="pscr")
                gall = sml.tile([128, 312], bf16, tag="gout")
                hmxa = ps.tile([128, 1536], f32, tag="mega")
                for it in range(NT):
                    for jt in range(NT):
                        nc.tensor.matmul(
                            hmxa[:, it * 512 : it * 512 + 300],
                            cts16[:, jt * S + it * 128
                                  : jt * S + (it + 1) * 128],
                            hmrhs[:, jt * 304 : jt * 304 + 300],
                            start=(jt == 0), stop=(jt == NT - 1))
                    nc.gpsimd.indirect_dma_start(
                        out=gall[:, it * 104 : (it + 1) * 104],
                        out_offset=None, in_=GT.ap(),
                        in_offset=bass.IndirectOffsetOnAxis(
                            ap=idx8[it][:, 0:1], axis=0))
                # hm (raw) -> next to pT for the m3 d3 products
                nc.vector.tensor_copy(ap3(ptsa, H, [[200, 3], [1, H]]),
                                      ap3(hmxa, 0, [[512, 3], [1, H]]))
                # m3 numerator/denominator products, all its at once
                nc.vector.tensor_tensor(
                    ap3(pscr, 0, [[400, 3], [1, 2 * H]]),
                    ap3(hmxa, H, [[512, 3], [1, 2 * H]]),
                    ap3(ptsa, 0, [[200, 3], [0, 2], [1, H]]), op=ALU.mult)
                nc.vector.tensor_tensor(
                    ap3(pscr, 2 * H, [[400, 3], [1, 2 * H]]),
                    ap3(hmxa, H, [[512, 3], [1, 2 * H]]),
                    ap3(ptsa, H, [[200, 3], [0, 2], [1, H]]), op=ALU.mult)
                nc.vector.reduce_sum(
                    fscap(0, [[16, 3], [1, 2]]),
                    ap3(pscr, 0, [[400, 3], [100, 2], [1, H]]), axis=AX.X)
                nc.vector.reduce_sum(
                    fscap(4, [[16, 3], [1, 2]]),
                    ap3(pscr, 2 * H, [[400, 3], [100, 2], [1, H]]),
                    axis=AX.X)
                # m4 products: hmaxT (read twice) * [p4uT_0 | p4uT_1]
                nc.vector.tensor_tensor(
                    pscr[:, 0:600],
                    ap3(gall, 0, [[104, 3], [0, 2], [1, H]]),
                    p4ua[:], op=ALU.mult)
                nc.vector.reduce_sum(
                    fscap(2, [[16, 3], [1, 2]]),
                    ap3(pscr, 0, [[200, 3], [100, 2], [1, H]]), axis=AX.X)
                nc.vector.tensor_copy(
                    fscap(6, [[16, 3], [1, 2]]),
                    ap3(gall, H, [[104, 3], [1, 2]]))

        # ---- phase E: finalize all pairs at once ---------------------
        # sign(sum_att) for m3 (the 1/sum magnitude cancels in the cosine)
        nc.scalar.activation(
            ap3(fscall, 13, [[48, NPAIR], [16, 3], [1, 1]]),
            ap3(fscall, 14, [[48, NPAIR], [16, 3], [1, 1]]), AF.Sign)
        # 1/sqrt over the m3/m4 denominators (cols +4..+6 per it block)
        den = ap3(fscall, 4, [[48, NPAIR], [16, 3], [1, 2]])
        nc.scalar.activation(den, den, AF.Sqrt)
        nc.vector.reciprocal(den, den)
        # m3/m4: tmp34 = num * invden, then * transposed 1/|w p| norms
        nc.vector.tensor_tensor(
            tmp34[:], ap3(fscall, 0, [[48, NPAIR], [16, 3], [1, 4]]),
            ap3(fscall, 4, [[48, NPAIR], [16, 3], [1, 4]]), op=ALU.mult)
        nc.vector.tensor_tensor(
            ap3(tmp34, 0, [[12, NPAIR], [4, 3], [1, 2]]),
            ap3(tmp34, 0, [[12, NPAIR], [4, 3], [1, 2]]),
            ap3(fscall, 13, [[48, NPAIR], [16, 3], [0, 2]]), op=ALU.mult)
        for di in range(2):
            for o in range(2):  # o=0: m3, o=1: m4
                nc.vector.tensor_tensor(
                    ap3(stgt, (4 + di) * 48 + 96 * o,
                        [[2, BC], [16, 3], [1, 2]]),
                    ap3(tmp34, di * BC * 12 + o * 2,
                        [[12, BC], [4, 3], [1, 2]]),
                    ap3(ipnall, di * BC * 27 + (7 if o == 0 else 1),
                        [[27, BC], [9, 3], [1, 2]]),
                    op=ALU.mult)
        # m1/m2: tmp12 = num * 1/|w p|, then m1 *= 1/nq
        for di in range(2):
            for o in range(2):  # o=0: m1, o=1: m2
                nc.vector.tensor_tensor(
                    ap3(tmp12, di * BC * 12 + o * 2,
                        [[12, BC], [4, 3], [1, 2]]),
                    ap3(fscall, di * BC * 48 + (11 if o == 0 else 8),
                        [[48, BC], [16, 3], [1, 2]]),
                    ap3(ipnall, di * BC * 27 + (5 if o == 0 else 3),
                        [[27, BC], [9, 3], [1, 2]]),
                    op=ALU.mult)
                nc.vector.tensor_tensor(
                    ap3(stgt, di * 48 + 96 * o, [[2, BC], [16, 3], [1, 2]]),
                    ap3(tmp12, di * BC * 12 + o * 2,
                        [[12, BC], [4, 3], [1, 2]]),
                    ap3(nq4all, di * BC * 4 + (0 if o == 0 else 2),
                        [[4, BC], [0, 3], [1, 2]]),
                    op=ALU.mult)

        for oi, nm in enumerate(("m1f", "m1b", "m2f", "m2b",
                                 "m3f", "m3b", "m4f", "m4b")):
            t = outs[nm]
            out_ap = AP(t, 0, [[16, 128], [2048, 3], [1, 16]])
            in_ap = ap3(stgt, oi * 48, [[16, 3], [1, 16]])
            nc.sync.dma_start(out_ap, in_ap)

        for p in (ps, stg, sml, big, con):
            p.release()

    nc.compile()
    return nc


def _host_consts(w1, w2, w3f_, w4f_):
    u1 = (w1 * w1).astype(np.float32)
    u2 = (w2 * w2).astype(np.float32)
    u3 = (w3f_ * w3f_).astype(np.float32)
    u4 = (w4f_ * w4f_).astype(np.float32)
    ones = np.ones((H,), np.float32)
    wsb = np.stack([ones, u4[0], u4[1], u2[0], u2[1],
                    u1[0], u1[1], u3[0], u3[1]], axis=1)  # (H, 9)
    return u1, u2, u3, u4, wsb


def _prepare(p_f, p_b, q_f, q_b, w1f, w1b, w2f, w2b, w3f, w3b, w4f, w4b):
    import ml_dtypes

    bfd = ml_dtypes.bfloat16
    if "prog" not in _COMPILED:
        _COMPILED["prog"] = _build_program()
    nc = _COMPILED["prog"]

    p_f, p_b = np.asarray(p_f), np.asarray(p_b)
    q_f, q_b = np.asarray(q_f), np.asarray(q_b)
    u1f, u2f_, u3, u4, wsbf = _host_consts(
        np.asarray(w1f), np.asarray(w2f), np.asarray(w3f), np.asarray(w4f))
    u1b, u2b_, _, _, wsbb = _host_consts(
        np.asarray(w1b), np.asarray(w2b), np.asarray(w3f), np.asarray(w4f))
    WSB = np.concatenate([wsbf, wsbb], axis=1).astype(np.float32)
    U1C = np.stack([u1f[0], u1f[1], u1b[0], u1b[1]], 1).astype(np.float32)
    U2C = np.stack([u2f_[0], u2f_[1], u2b_[0], u2b_[1]], 1).astype(np.float32)
    ubc = np.concatenate([u3[0], u3[1], u4[0], u4[1]]).astype(np.float32)
    UBC16 = np.ascontiguousarray(
        np.broadcast_to(ubc, (128, 400))).astype(bfd)
    SEL16 = np.zeros((9, 384), np.float32)
    SEL16[3, 128:256] = 1.0  # invq2_0
    SEL16[4, 256:384] = 1.0  # invq2_1
    IDN = np.eye(128, dtype=np.float32)

    in_maps = []
    for c in range(NCORES):
        sl = slice(c * BC, (c + 1) * BC)
        in_maps.append({
            "p_f": np.ascontiguousarray(p_f[:, sl, :]),
            "p_b": np.ascontiguousarray(p_b[:, sl, :]),
            "q_f": np.ascontiguousarray(q_f[:, sl, :]),
            "q_b": np.ascontiguousarray(q_b[:, sl, :]),
            "WSB16": WSB.astype(bfd), "WSBF": WSB,
            "U1C": U1C, "U2C": U2C, "UBC16": UBC16,
            "SEL16": SEL16.astype(bfd),
            "IDN": IDN, "IDN16": IDN.astype(bfd),
        })

    return nc, in_maps


def _gather_outputs(results):
    full = []
    for nm in ("m1f", "m1b", "m2f", "m2b", "m3f", "m3b", "m4f", "m4b"):
        full.append(np.concatenate([results[c][nm] for c in range(NCORES)],
                                   axis=1))
    return tuple(full)


def kernel(**inputs):
    from concourse.bass_utils import run_bass_kernel_spmd

    nc, in_maps = _prepare(**inputs)
    res = run_bass_kernel_spmd(nc, in_maps, list(range(NCORES)))
    return _gather_outputs(res.results)


def run_traced(**inputs):
    """Run with NTFF profiling; returns (exec_time_ns, results_obj)."""
    from concourse.bass_utils import run_bass_kernel_spmd

    nc, in_maps = _prepare(**inputs)
    res = run_bass_kernel_spmd(nc, in_maps, list(range(NCORES)), trace=True)
    return res.exec_time_ns, res
